# revision 1
# baseline (speedup 1.0000x reference)
"""CombinedRotaryEmbedding Trainium2 kernel.

Math (per 64-dim head, per position s):
    xh  = x @ R                 (R = composed Givens rotations @ rotation_matrix)
    u   = xh[..., 0::2]  = x @ R[:, 0::2]
    v   = xh[..., 1::2]  = x @ R[:, 1::2]
    out = [u*cos - v*sin | u*sin + v*cos]      cos/sin = f(position, freq[32])

Kernel strategy (8-way data parallel over the sequence dim):
  - host: compose R (tiny [64,64]), build R2 = [R_even | R_odd]; build per-core
    cos/sin tables CC = [cos|cos], SS = [sin|-sin] laid out per SBUF partition.
  - device, per core (x shard [2048 rows, 1024]):
      PE   : transpose x [128,128] chunks (feat -> partitions)
      ACT  : copy transposed chunks PSUM -> SBUF
      PE   : per head, y2[128 rows, 64] = xT_head.T @ R2   ([u|v] layout)
      DVE  : t1 = y2*CC, t2 = y2*SS   (PSUM -> SBUF)
      GPSIMD: out_lo = t1_lo + t2_hi ; out_hi = t1_hi + t2_lo
      DMA  : contiguous row-tile loads/stores (nc.sync HWDGE)
"""

import numpy as np

import concourse.bacc as bacc
import concourse.bass as bass
import concourse.tile as tile
from concourse import mybir
from concourse.bass_utils import run_bass_kernel_spmd
from concourse.masks import make_identity

N_CORES = 8
B, S, N_STATE = 4, 4096, 1024
H, D = 16, 64           # heads, head dim
HALF = D // 2           # 32 rotary freqs
S_SH = S // N_CORES     # 512 positions per core
ROWS = B * S_SH         # 2048 rows of [1024] per core
RT = ROWS // 128        # 16 row tiles
CBLK = S_SH // 128      # 4 distinct position blocks per core
F32 = mybir.dt.float32

_compiled = {}


def _build_nc(adds_on="gpsimd", xin_bufs=4, outp_bufs=4, ysb_bufs=4,
              tmp_bufs=6, xtp_bufs=2, tpsum_bufs=2, ypsum_bufs=3,
              out_split=2, ablate="full"):
    nc = bacc.Bacc("TRN2")
    x_in = nc.dram_tensor("x", [ROWS, N_STATE], F32, kind="ExternalInput")
    # r2 arrives as blockdiag(R2, R2) so one K=128 matmul covers 2 heads
    r2_in = nc.dram_tensor("r2", [128, 128], F32, kind="ExternalInput")
    # ccss[p, c, 0] = [cos|cos], ccss[p, c, 1] = [sin|-sin] for position c*128+p
    ccss_in = nc.dram_tensor("ccss", [128, CBLK, 2, D], F32, kind="ExternalInput")
    out_d = nc.dram_tensor("out", [ROWS, N_STATE], F32, kind="ExternalOutput")

    with tile.TileContext(nc) as tc:
        with (
            tc.tile_pool(name="const", bufs=1) as const,
            tc.tile_pool(name="xin", bufs=xin_bufs) as xin,
            tc.tile_pool(name="xtp", bufs=xtp_bufs) as xtp,
            tc.tile_pool(name="tpsum", bufs=tpsum_bufs, space="PSUM") as tpsum,
            tc.tile_pool(name="ypsum", bufs=ypsum_bufs, space="PSUM") as ypsum,
            tc.tile_pool(name="ysb", bufs=ysb_bufs) as ysb,
            tc.tile_pool(name="tmp", bufs=tmp_bufs) as tmp,
            tc.tile_pool(name="outp", bufs=outp_bufs) as outp,
        ):
            ident = const.tile([128, 128], F32)
            make_identity(nc, ident[:])
            r2_sb = const.tile([128, 128], F32)
            nc.sync.dma_start(out=r2_sb[:], in_=r2_in[:])
            # ccss_sb[p, c*2+t, 0:128] = ccss_in[p, c, t, :] duplicated twice
            # (DMA re-reads the 64-wide row via a step-0 dim)
            ccss_sb = const.tile([128, CBLK * 2, 2, D], F32)
            ccss_src = ccss_in[:]
            nc.sync.dma_start(
                out=ccss_sb[:],
                in_=bass.AP(
                    tensor=ccss_src.tensor, offset=ccss_src.offset,
                    ap=[list(ccss_src.ap[0]), [D, CBLK * 2], [0, 2], [1, D]],
                ),
            )

            for rt in range(RT):
                c = rt % CBLK
                x_t = xin.tile([128, N_STATE], F32)
                nc.sync.dma_start(out=x_t[:], in_=x_in[rt * 128:(rt + 1) * 128, :])

                if ablate == "dma":
                    ot = outp.tile([128, N_STATE], F32, tag="otd")
                    nc.vector.tensor_copy(ot[:], x_t[:])
                    nc.sync.dma_start(
                        out=out_d[rt * 128:(rt + 1) * 128], in_=ot[:])
                    continue

                # transpose 8 x [128,128] chunks; feats end up on partitions
                xT = xtp.tile([128, 8, 128], F32)
                for g in range(2):
                    tp = tpsum.tile([128, 4, 128], F32)
                    for q in range(4):
                        ch = g * 4 + q
                        nc.tensor.transpose(
                            tp[:, q, :],
                            x_t[:, ch * 128:(ch + 1) * 128],
                            ident[:],
                        )
                    nc.scalar.copy(out=xT[:, g * 4:(g + 1) * 4, :], in_=tp[:])

                # out_t [p, g2, j, b, e]: head = g2*4 + 2j + b, flat = natural
                out_t = outp.tile([128, 4, 2, 2, D], F32)
                for g2 in range(4):
                    # one matmul per PSUM bank (HW: >1 matmul/bank is fatal);
                    # each matmul computes 2 heads via the block-diagonal rhs
                    y2p = ypsum.tile([128, 2, 512], F32)
                    for j in range(2):
                        nc.tensor.matmul(
                            y2p[:, j, 0:128], xT[:, g2 * 2 + j, :], r2_sb[:],
                            start=True, stop=True,
                        )
                    # one fused DVE op: t12[t] = y2 * (cos-row if t==0 else sin-row)
                    # in0 doubles the psum read via a step-0 leading dim
                    y2ap = y2p[:, :, 0:128]
                    t12 = tmp.tile([128, 2, 2, 128], F32, tag="t12")
                    nc.vector.tensor_mul(
                        t12[:],
                        bass.AP(tensor=y2ap.tensor, offset=y2ap.offset,
                                ap=[list(y2ap.ap[0]), [0, 2], [512, 2],
                                    [1, 128]]),
                        bass.AP(tensor=ccss_sb.tensor,
                                offset=ccss_sb[:].offset + c * 256,
                                ap=[list(ccss_sb[:].ap[0]), [128, 2], [0, 2],
                                    [1, 128]]),
                    )
                    # one fused gpsimd op: crossed add, in1 reads swapped
                    # 32-halves via a negative mid-stride
                    og = out_t[:, g2]
                    t12a = t12[:]
                    eng = nc.gpsimd if adds_on == "gpsimd" else nc.vector
                    eng.tensor_tensor(
                        out=bass.AP(tensor=og.tensor, offset=og.offset,
                                    ap=[list(og.ap[0]), [D, 4], [HALF, 2],
                                        [1, HALF]]),
                        in0=bass.AP(tensor=t12a.tensor, offset=t12a.offset,
                                    ap=[list(t12a.ap[0]), [D, 4], [HALF, 2],
                                        [1, HALF]]),
                        in1=bass.AP(tensor=t12a.tensor,
                                    offset=t12a.offset + 256 + HALF,
                                    ap=[list(t12a.ap[0]), [D, 4], [-HALF, 2],
                                        [1, HALF]]),
                        op=mybir.AluOpType.add,
                    )
                    if out_split > 1 and g2 % (4 // out_split) == (4 // out_split) - 1:
                        w = N_STATE // out_split
                        s = g2 // (4 // out_split)
                        flat = out_t[:].rearrange("p a b c d -> p (a b c d)")
                        nc.sync.dma_start(
                            out=out_d[rt * 128:(rt + 1) * 128,
                                      s * w:(s + 1) * w],
                            in_=flat[:, s * w:(s + 1) * w])
                flat = out_t[:].rearrange("p a b c d -> p (a b c d)")
                if out_split == 1:
                    nc.sync.dma_start(
                        out=out_d[rt * 128:(rt + 1) * 128], in_=flat)
    nc.compile()  # bacc: splits multi-sem waits into EventSemaphore insts
    return nc


def _compose_r2(thetas, rotation_pairs, theta_scale, rotation_matrix):
    """Replicates reference._compose_rotation, then permutes cols to [even|odd]."""
    idx = rotation_pairs.astype(np.int32)
    th = (thetas.astype(np.float32) * np.float32(theta_scale[0]))
    R = np.eye(D, dtype=np.float32)
    for k in range(th.shape[0]):
        i, j = int(idx[k, 0]), int(idx[k, 1])
        ck, sk = np.float32(np.cos(th[k])), np.float32(np.sin(th[k]))
        G = np.eye(D, dtype=np.float32)
        G[i, i] = ck
        G[i, j] = -sk
        G[j, i] = sk
        G[j, j] = ck
        R = (R @ G).astype(np.float32)
    R = (R @ rotation_matrix.astype(np.float32)).astype(np.float32)
    return np.ascontiguousarray(
        np.concatenate([R[:, 0::2], R[:, 1::2]], axis=1), dtype=np.float32
    )


def _pos_tables(inv_freq):
    pos = np.arange(S, dtype=np.float32)
    sinusoid = pos[:, None] * inv_freq[None, :].astype(np.float32)  # [S, 32]
    return np.cos(sinusoid).astype(np.float32), np.sin(sinusoid).astype(np.float32)


def _ccss_layout(cos_blk, sin_blk):
    """[512, 32] cos/sin tables -> [128 part, CBLK, 2, 64] compact layout:
    row t=0 is [cos|cos], row t=1 is [sin|-sin], partition = pos % 128."""
    cc = np.concatenate([cos_blk, cos_blk], axis=1)           # [512, 64]
    ss = np.concatenate([sin_blk, -sin_blk], axis=1)
    t = np.stack([cc, ss], axis=1)                            # [512, 2, 64]
    t = t.reshape(CBLK, 128, 2, D).transpose(1, 0, 2, 3)      # [128, CBLK, 2, 64]
    return np.ascontiguousarray(t, dtype=np.float32)


def make_in_maps(x, thetas, rotation_pairs, theta_scale, rotation_matrix,
                 inv_freq):
    x = np.asarray(x, dtype=np.float32)
    r2s = _compose_r2(
        np.asarray(thetas, np.float32),
        np.asarray(rotation_pairs, np.float32),
        np.asarray(theta_scale, np.float32),
        np.asarray(rotation_matrix, np.float32),
    )
    r2 = np.zeros((128, 128), dtype=np.float32)
    r2[0:D, 0:D] = r2s
    r2[D:128, D:128] = r2s
    cosf, sinf = _pos_tables(np.asarray(inv_freq, np.float32))

    in_maps = []
    for k in range(N_CORES):
        blk = slice(k * S_SH, (k + 1) * S_SH)
        ccss = _ccss_layout(cosf[blk], sinf[blk])
        xs = np.ascontiguousarray(x[:, blk, :]).reshape(ROWS, N_STATE)
        in_maps.append({"x": xs, "r2": r2, "ccss": ccss})
    return in_maps


def kernel(x, thetas, rotation_pairs, theta_scale, rotation_matrix, inv_freq):
    in_maps = make_in_maps(x, thetas, rotation_pairs, theta_scale,
                           rotation_matrix, inv_freq)
    if "nc" not in _compiled:
        _compiled["nc"] = _build_nc()
    res = run_bass_kernel_spmd(_compiled["nc"], in_maps, list(range(N_CORES))).results

    out = np.empty((B, S, N_STATE), dtype=np.float32)
    for k in range(N_CORES):
        blk = slice(k * S_SH, (k + 1) * S_SH)
        out[:, blk, :] = res[k]["out"].reshape(B, S_SH, N_STATE)
    return out



# revision 15
# speedup vs baseline: 1.2543x; 1.2543x over previous
"""CombinedRotaryEmbedding Trainium2 kernel.

Math (per 64-dim head, per position s, with R = composed Givens @ rotation_matrix):
    u = x @ R[:, 0::2],  v = x @ R[:, 1::2]
    out = [u*cos - v*sin | u*sin + v*cos]     cos/sin = f(position, freq[32])

Restructured as:  out = P .* CC + Q .* SS   with
    P = x @ [R_even | R_odd]        ([u|v])
    Q = x @ [-R_odd | R_even]       ([-v|u])
    CC = [cos|cos],  SS = [sin|sin]   (element-aligned halves, no swap needed)
so one fused matmul per 2-head chunk produces [P|Q] (free dim 256 -> float32r
runs at 1 cycle/row), DVE does one multiply pass, gpsimd one add pass.

Kernel strategy (8-way data parallel over the sequence dim):
  - host: tiny tables only: RR = [blockdiag(R2s,R2s) | blockdiag(R2sw,R2sw)]
    [128,256] and compact ccss [128, 4, 2, 32] (cos/sin rows per position).
  - device, per core (x shard [2048 rows, 1024] = 8 blocks of 256 rows):
      SP   : all 8 x-block loads issued upfront (1 MiB each), stores trail
      PE   : transpose via bf16 identity (1 cyc/row), [P|Q] matmuls in f32r
      ACT  : copy transposed chunks PSUM -> SBUF
      DVE  : t = [P|Q] * [CC|SS]  (PSUM -> SBUF, one op per 2-chunk pair)
      GPSIMD: out = t_lo + t_hi   (one op per 128-row subtile)
"""

import numpy as np

import concourse.bacc as bacc
import concourse.bass as bass
import concourse.tile as tile
from concourse import mybir
from concourse.bass_utils import run_bass_kernel_spmd

N_CORES = 8
B, S, N_STATE = 4, 4096, 1024
H, D = 16, 64            # heads, head dim
HALF = D // 2            # 32 rotary freqs
S_SH = S // N_CORES      # 512 positions per core
ROWS = B * S_SH          # 2048 rows of [1024] per core
NBLK = ROWS // 256       # 8 DMA blocks of 256 rows
CBLK = S_SH // 128       # 4 distinct position blocks per core
NCH = N_STATE // 128     # 8 two-head chunks per row
F32 = mybir.dt.float32
F32R = mybir.dt.float32r
BF16 = mybir.dt.bfloat16

_compiled = {}


def _build_nc():
    nc = bacc.Bacc("TRN2")
    x_in = nc.dram_tensor("x", [ROWS, N_STATE], F32R, kind="ExternalInput")
    ident_in = nc.dram_tensor("ident", [128, 128], F32R, kind="ExternalInput")
    # RR = [blockdiag(R2s,R2s) | blockdiag(R2sw,R2sw)]: one K=128 matmul
    # yields [P|Q] for 2 heads
    rr_in = nc.dram_tensor("rr", [128, 256], F32R, kind="ExternalInput")
    # compact per-position rows: ccss[p, c, 0] = cos32, ccss[p, c, 1] = sin32
    ccss_in = nc.dram_tensor("ccss", [128, CBLK, 2, HALF], F32,
                             kind="ExternalInput")
    out_d = nc.dram_tensor("out", [ROWS, N_STATE], F32, kind="ExternalOutput")

    with tile.TileContext(nc) as tc:
        with (
            tc.tile_pool(name="const", bufs=1) as const,
            tc.tile_pool(name="xin", bufs=2 * NBLK) as xin,
            tc.tile_pool(name="xtp", bufs=3) as xtp,
            tc.tile_pool(name="tpsum", bufs=2, space="PSUM") as tpsum,
            tc.tile_pool(name="ypsum", bufs=2, space="PSUM") as ypsum,
            tc.tile_pool(name="tsb", bufs=3) as tsb,
            # one buffer per subtile: output buffers must never gate compute
            # (stores drain late because loads occupy the DMA engines first)
            tc.tile_pool(name="outp", bufs=2 * NBLK) as outp,
        ):
            ident = const.tile([128, 128], F32R)
            nc.sync.dma_start(out=ident[:], in_=ident_in[:])

            # first subtile load goes ahead of everything else, split in two
            # so the transpose->copy->matmul->mul chain starts as early as
            # possible (chunks 0-3 usable after a 256 KB transfer)
            xts = []
            x_t = xin.tile([128, N_STATE], F32R)
            nc.sync.dma_start(out=x_t[:, 0:512], in_=x_in[0:128, 0:512])
            xts.append(x_t)

            rr_sb = const.tile([128, 256], F32R)
            nc.sync.dma_start(out=rr_sb[:], in_=rr_in[:])
            ccss_c = const.tile([128, CBLK, 2, HALF], F32)
            nc.sync.dma_start(out=ccss_c[:], in_=ccss_in[:])
            nc.sync.dma_start(out=x_t[:, 512:1024], in_=x_in[0:128, 512:1024])
            # expand to [128, c, 2, 128]: row t repeated 4x along the free
            # dim; c=0 on DVE (gates the very first mul), rest on gpsimd to
            # keep them off the DVE's critical stream
            ccss_sb = const.tile([128, CBLK, 2, 128], F32)
            cbase = ccss_sb[:]
            sbase = ccss_c[:]
            for c in range(CBLK):
                eng = nc.vector if c == 0 else nc.gpsimd
                eng.tensor_copy(
                    bass.AP(tensor=cbase.tensor, offset=cbase.offset + c * 256,
                            ap=[list(cbase.ap[0]), [128, 2], [HALF, 4],
                                [1, HALF]]),
                    bass.AP(tensor=sbase.tensor, offset=sbase.offset + c * 64,
                            ap=[list(sbase.ap[0]), [HALF, 2], [0, 4],
                                [1, HALF]]),
                )

            # remaining subtile loads, all upfront: store waits can never
            # delay a load (single SP queue, loads issued first)
            for st in range(1, 2 * NBLK):
                x_t = xin.tile([128, N_STATE], F32R)
                nc.sync.dma_start(out=x_t[:],
                                  in_=x_in[st * 128:(st + 1) * 128, :])
                xts.append(x_t)

            for blk in range(NBLK):
                for j in range(2):
                    x_t = xts[2 * blk + j]
                    c = (2 * blk + j) % CBLK
                    xT = xtp.tile([128, NCH, 128], F32R)

                    def transpose_group(g):
                        # transpose 4 chunks: feats -> partitions
                        tp = tpsum.tile([128, 4, 128], F32R)
                        for q in range(4):
                            ch = 4 * g + q
                            nc.tensor.transpose(
                                tp[:, q, :],
                                x_t[:, ch * 128:(ch + 1) * 128],
                                ident[:],
                            )
                        nc.scalar.copy(out=xT[:, 4 * g:4 * (g + 1), :],
                                       in_=tp[:])

                    transpose_group(0)
                    t_sb = tsb.tile([128, NCH, 256], F32)
                    tb = t_sb[:]
                    o_t = outp.tile([128, N_STATE], F32)
                    ob = o_t[:]
                    adds_done = 0
                    for g0, gn in ((0, 3), (3, 3), (6, 2)):
                        if g0 == 3:
                            # second transpose group goes after the first
                            # matmul group so PE unblocks DVE's group-0 mul
                            # as early as possible
                            transpose_group(1)
                        # [P|Q] per chunk; one matmul per PSUM bank
                        pq = ypsum.tile([128, 3, 512], F32)
                        for a in range(gn):
                            nc.tensor.matmul(
                                pq[:, a, 0:256], xT[:, g0 + a, :], rr_sb[:],
                                start=True, stop=True,
                            )
                        pqb = pq[:]
                        nc.vector.tensor_mul(
                            bass.AP(tensor=tb.tensor,
                                    offset=tb.offset + g0 * 256,
                                    ap=[list(tb.ap[0]), [256, gn], [1, 256]]),
                            bass.AP(tensor=pqb.tensor, offset=pqb.offset,
                                    ap=[list(pqb.ap[0]), [512, gn], [1, 256]]),
                            bass.AP(tensor=cbase.tensor,
                                    offset=cbase.offset + c * 256,
                                    ap=[list(cbase.ap[0]), [0, gn], [128, 2],
                                        [1, 128]]),
                        )
                        # half-subtile ready (chunks 4h..4h+3): add + store so
                        # the tail after the last mul is only half a subtile
                        while (g0 + gn) >= 4 * (adds_done + 1):
                            h = adds_done
                            # the very last add is pure tail (after the final
                            # mul): DVE is faster and skips Pool's serial queue
                            last = (blk == NBLK - 1 and j == 1 and h == 1)
                            eng = nc.vector if last else nc.gpsimd
                            eng.tensor_tensor(
                                out=bass.AP(tensor=ob.tensor,
                                            offset=ob.offset + h * 512,
                                            ap=[list(ob.ap[0]), [128, 4],
                                                [1, 128]]),
                                in0=bass.AP(tensor=tb.tensor,
                                            offset=tb.offset + h * 1024,
                                            ap=[list(tb.ap[0]), [256, 4],
                                                [1, 128]]),
                                in1=bass.AP(tensor=tb.tensor,
                                            offset=tb.offset + h * 1024 + 128,
                                            ap=[list(tb.ap[0]), [256, 4],
                                                [1, 128]]),
                                op=mybir.AluOpType.add,
                            )
                            r0 = blk * 256 + j * 128
                            nc.sync.dma_start(
                                out=out_d[r0:r0 + 128, h * 512:(h + 1) * 512],
                                in_=o_t[:, h * 512:(h + 1) * 512])
                            adds_done += 1
    nc.compile()  # bacc: splits multi-sem waits into EventSemaphore insts
    return nc


def _compose_r(thetas, rotation_pairs, theta_scale, rotation_matrix):
    """Replicates reference._compose_rotation."""
    idx = rotation_pairs.astype(np.int32)
    th = thetas.astype(np.float32) * np.float32(theta_scale[0])
    R = np.eye(D, dtype=np.float32)
    for k in range(th.shape[0]):
        i, j = int(idx[k, 0]), int(idx[k, 1])
        ck, sk = np.float32(np.cos(th[k])), np.float32(np.sin(th[k]))
        G = np.eye(D, dtype=np.float32)
        G[i, i] = ck
        G[i, j] = -sk
        G[j, i] = sk
        G[j, j] = ck
        R = (R @ G).astype(np.float32)
    return (R @ rotation_matrix.astype(np.float32)).astype(np.float32)


def _build_rr(R):
    """[128, 256] = [blockdiag(R2s,R2s) | blockdiag(R2sw,R2sw)] where
    R2s = [R_even|R_odd] (-> P = [u|v]) and R2sw = [-R_odd|R_even]
    (-> Q = [-v|u])."""
    r2s = np.concatenate([R[:, 0::2], R[:, 1::2]], axis=1)
    r2sw = np.concatenate([-R[:, 1::2], R[:, 0::2]], axis=1)

    def blkdiag(m):
        z = np.zeros((128, 128), dtype=np.float32)
        z[0:D, 0:D] = m
        z[D:128, D:128] = m
        return z

    return np.ascontiguousarray(
        np.concatenate([blkdiag(r2s), blkdiag(r2sw)], axis=1), dtype=np.float32)


def make_in_maps(x, thetas, rotation_pairs, theta_scale, rotation_matrix,
                 inv_freq):
    x = np.asarray(x, dtype=np.float32)
    R = _compose_r(
        np.asarray(thetas, np.float32),
        np.asarray(rotation_pairs, np.float32),
        np.asarray(theta_scale, np.float32),
        np.asarray(rotation_matrix, np.float32),
    )
    rr = _build_rr(R)
    invf = np.asarray(inv_freq, np.float32)
    pos = np.arange(S, dtype=np.float32)
    sinusoid = pos[:, None] * invf[None, :]               # [S, 32]
    cosf = np.cos(sinusoid).astype(np.float32)
    sinf = np.sin(sinusoid).astype(np.float32)

    in_maps = []
    for k in range(N_CORES):
        blk = slice(k * S_SH, (k + 1) * S_SH)
        # ccss[p, c, 0] = cos row, ccss[p, c, 1] = sin row (pos = c*128 + p)
        cc = cosf[blk].reshape(CBLK, 128, HALF)
        ss = sinf[blk].reshape(CBLK, 128, HALF)
        ccss = np.ascontiguousarray(
            np.stack([cc, ss], axis=2).transpose(1, 0, 2, 3), dtype=np.float32)
        xs = np.ascontiguousarray(x[:, blk, :]).reshape(ROWS, N_STATE)
        in_maps.append({"x": xs, "rr": rr, "ccss": ccss,
                        "ident": np.eye(128, dtype=np.float32)})
    return in_maps


def kernel(x, thetas, rotation_pairs, theta_scale, rotation_matrix, inv_freq):
    in_maps = make_in_maps(x, thetas, rotation_pairs, theta_scale,
                           rotation_matrix, inv_freq)
    if "nc" not in _compiled:
        _compiled["nc"] = _build_nc()
    res = run_bass_kernel_spmd(_compiled["nc"], in_maps,
                               list(range(N_CORES))).results

    out = np.empty((B, S, N_STATE), dtype=np.float32)
    for k in range(N_CORES):
        blk = slice(k * S_SH, (k + 1) * S_SH)
        out[:, blk, :] = res[k]["out"].reshape(B, S_SH, N_STATE)
    return out


# revision 33
# speedup vs baseline: 1.2608x; 1.0052x over previous
"""CombinedRotaryEmbedding Trainium2 kernel.

Math (per 64-dim head, per position s, with R = composed Givens @ rotation_matrix):
    u = x @ R[:, 0::2],  v = x @ R[:, 1::2]
    out = [u*cos - v*sin | u*sin + v*cos]     cos/sin = f(position, freq[32])

Restructured as:  out = P .* CC + Q .* SS   with
    P = x @ [R_even | R_odd]        ([u|v])
    Q = x @ [-R_odd | R_even]       ([-v|u])
    CC = [cos|cos],  SS = [sin|sin]   (element-aligned halves, no swap needed)
so one fused matmul per 2-head chunk produces [P|Q] (free dim 256 -> float32r
runs at 1 cycle/row), DVE does one multiply pass, gpsimd one add pass.

Kernel strategy (8-way data parallel over the sequence dim):
  - host: tiny tables only: RR = [blockdiag(R2s,R2s) | blockdiag(R2sw,R2sw)]
    [128,256], compact ccss [128, 4, 2, 32] (cos/sin rows per position),
    and a 128x128 identity for the PE transposes.
  - device, per core (x shard [2048 rows, 1024] = 16 subtiles of 128 rows):
      SP   : all 16 subtile loads issued upfront (512 KB each), then stores;
             loads first means a store's sem wait can never delay a load
      PE   : transpose x chunks via f32r identity (1.5 cyc/row), then one
             f32r matmul per 2-head chunk -> [P|Q] (256 free, 1 cyc/row)
      ACT  : copy transposed chunks PSUM -> SBUF (4-chunk groups)
      DVE  : t = [P|Q] * [CC|SS]  (PSUM -> SBUF, [3,3,2]-chunk groups,
             sized by the 6 PSUM banks left next to the transpose tiles)
      GPSIMD: out = t_lo + t_hi per half-subtile, stored immediately
    The DVE multiply stream is the pacer (~41 us busy, gapless); the final
    subtile uses quarter-granularity adds/stores (last one on DVE) to
    shorten the tail after the last multiply.
"""

import numpy as np

import concourse.bacc as bacc
import concourse.bass as bass
import concourse.tile as tile
from concourse import mybir
from concourse.bass_utils import run_bass_kernel_spmd

N_CORES = 8
B, S, N_STATE = 4, 4096, 1024
H, D = 16, 64            # heads, head dim
HALF = D // 2            # 32 rotary freqs
S_SH = S // N_CORES      # 512 positions per core
ROWS = B * S_SH          # 2048 rows of [1024] per core
NBLK = ROWS // 256       # 8 DMA blocks of 256 rows
CBLK = S_SH // 128       # 4 distinct position blocks per core
NCH = N_STATE // 128     # 8 two-head chunks per row
F32 = mybir.dt.float32
F32R = mybir.dt.float32r
BF16 = mybir.dt.bfloat16

_compiled = {}


def _build_nc():
    nc = bacc.Bacc("TRN2")
    x_in = nc.dram_tensor("x", [ROWS, N_STATE], F32R, kind="ExternalInput")
    ident_in = nc.dram_tensor("ident", [128, 128], F32R, kind="ExternalInput")
    # RR = [blockdiag(R2s,R2s) | blockdiag(R2sw,R2sw)]: one K=128 matmul
    # yields [P|Q] for 2 heads
    rr_in = nc.dram_tensor("rr", [128, 256], F32R, kind="ExternalInput")
    # compact per-position rows: ccss[p, c, 0] = cos32, ccss[p, c, 1] = sin32
    ccss_in = nc.dram_tensor("ccss", [128, CBLK, 2, HALF], F32,
                             kind="ExternalInput")
    out_d = nc.dram_tensor("out", [ROWS, N_STATE], F32, kind="ExternalOutput")

    with tile.TileContext(nc) as tc:
        with (
            tc.tile_pool(name="const", bufs=1) as const,
            tc.tile_pool(name="xin", bufs=2 * NBLK) as xin,
            tc.tile_pool(name="xtp", bufs=3) as xtp,
            tc.tile_pool(name="tpsum", bufs=2, space="PSUM") as tpsum,
            tc.tile_pool(name="ypsum", bufs=2, space="PSUM") as ypsum,
            tc.tile_pool(name="tsb", bufs=3) as tsb,
            # one buffer per subtile: output buffers must never gate compute
            # (stores drain late because loads occupy the DMA engines first)
            tc.tile_pool(name="outp", bufs=2 * NBLK) as outp,
        ):
            # first subtile load goes ahead of everything else, split in two
            # so the transpose->copy->matmul->mul chain starts as early as
            # possible (chunks 0-3 usable after a 256 KB transfer)
            xts = []
            x_t = xin.tile([128, N_STATE], F32R)
            nc.sync.dma_start(out=x_t[:, 0:512], in_=x_in[0:128, 0:512])
            xts.append(x_t)

            ident = const.tile([128, 128], F32R)
            nc.sync.dma_start(out=ident[:], in_=ident_in[:])
            ccss_c = const.tile([128, CBLK, 2, HALF], F32)
            nc.sync.dma_start(out=ccss_c[:], in_=ccss_in[:])
            rr_sb = const.tile([128, 256], F32R)
            nc.sync.dma_start(out=rr_sb[:], in_=rr_in[:])
            nc.sync.dma_start(out=x_t[:, 512:1024], in_=x_in[0:128, 512:1024])

            # expand to [128, c, 2, 128]: row t repeated 4x along the free
            # dim; c=0 on DVE (gates the very first mul), rest on gpsimd to
            # keep them off the DVE's critical stream
            ccss_sb = const.tile([128, CBLK, 2, 128], F32)
            cbase = ccss_sb[:]
            sbase = ccss_c[:]
            for c in range(CBLK):
                eng = nc.vector if c == 0 else nc.gpsimd
                eng.tensor_copy(
                    bass.AP(tensor=cbase.tensor, offset=cbase.offset + c * 256,
                            ap=[list(cbase.ap[0]), [128, 2], [HALF, 4],
                                [1, HALF]]),
                    bass.AP(tensor=sbase.tensor, offset=sbase.offset + c * 64,
                            ap=[list(sbase.ap[0]), [HALF, 2], [0, 4],
                                [1, HALF]]),
                )

            # remaining subtile loads, all upfront: store waits can never
            # delay a load (single SP queue, loads issued first)
            for st in range(1, 2 * NBLK):
                x_t = xin.tile([128, N_STATE], F32R)
                nc.sync.dma_start(out=x_t[:],
                                  in_=x_in[st * 128:(st + 1) * 128, :])
                xts.append(x_t)

            for blk in range(NBLK):
                for j in range(2):
                    x_t = xts[2 * blk + j]
                    c = (2 * blk + j) % CBLK
                    xT = xtp.tile([128, NCH, 128], F32R)
                    t_sb = tsb.tile([128, NCH, 256], F32)
                    tb = t_sb[:]
                    o_t = outp.tile([128, N_STATE], F32)
                    ob = o_t[:]
                    def transpose_group(g):
                        # transpose 4 chunks: feats -> partitions
                        tp = tpsum.tile([128, 4, 128], F32R, tag="tp")
                        for q in range(4):
                            ch = 4 * g + q
                            nc.tensor.transpose(
                                tp[:, q, :],
                                x_t[:, ch * 128:(ch + 1) * 128],
                                ident[:],
                            )
                        nc.scalar.copy(out=xT[:, 4 * g:4 * (g + 1), :],
                                       in_=tp[:])

                    def transpose_chunks(g0, gn):
                        # per-mul-group staging: shortest startup chain
                        tp = tpsum.tile([128, 4, 128], F32R, tag="tp")
                        for a in range(gn):
                            ch = g0 + a
                            nc.tensor.transpose(
                                tp[:, a, :],
                                x_t[:, ch * 128:(ch + 1) * 128],
                                ident[:],
                            )
                        nc.scalar.copy(out=xT[:, g0:g0 + gn, :],
                                       in_=tp[:, 0:gn, :])

                    first = (blk == 0 and j == 0)
                    if not first:
                        transpose_group(0)
                    adds_done = 0
                    for g0, gn in ((0, 3), (3, 3), (6, 2)):
                        if first:
                            transpose_chunks(g0, gn)
                        elif g0 == 3:
                            # second transpose group after the first matmul
                            # group: PE unblocks DVE's group-0 mul earliest
                            transpose_group(1)
                        # [P|Q] per chunk; one matmul per PSUM bank
                        pq = ypsum.tile([128, 3, 512], F32)
                        for a in range(gn):
                            nc.tensor.matmul(
                                pq[:, a, 0:256], xT[:, g0 + a, :], rr_sb[:],
                                start=True, stop=True,
                            )
                        pqb = pq[:]
                        nc.vector.tensor_mul(
                            bass.AP(tensor=tb.tensor,
                                    offset=tb.offset + g0 * 256,
                                    ap=[list(tb.ap[0]), [256, gn], [1, 256]]),
                            bass.AP(tensor=pqb.tensor, offset=pqb.offset,
                                    ap=[list(pqb.ap[0]), [512, gn], [1, 256]]),
                            bass.AP(tensor=cbase.tensor,
                                    offset=cbase.offset + c * 256,
                                    ap=[list(cbase.ap[0]), [0, gn], [128, 2],
                                        [1, 128]]),
                        )
                        # ready chunks -> add + store; half-subtile granules
                        # normally, quarters on the final subtile so the tail
                        # after the very last mul is as short as possible
                        final = (blk == NBLK - 1 and j == 1)
                        gran = 2 if final else 4
                        while (g0 + gn) >= gran * (adds_done + 1):
                            h = adds_done
                            # trailing adds skip Pool's serial queue via DVE
                            eng = nc.vector if (final and h == 3) else nc.gpsimd
                            w = gran * 128
                            eng.tensor_tensor(
                                out=bass.AP(tensor=ob.tensor,
                                            offset=ob.offset + h * w,
                                            ap=[list(ob.ap[0]), [128, gran],
                                                [1, 128]]),
                                in0=bass.AP(tensor=tb.tensor,
                                            offset=tb.offset + h * gran * 256,
                                            ap=[list(tb.ap[0]), [256, gran],
                                                [1, 128]]),
                                in1=bass.AP(tensor=tb.tensor,
                                            offset=tb.offset + h * gran * 256
                                            + 128,
                                            ap=[list(tb.ap[0]), [256, gran],
                                                [1, 128]]),
                                op=mybir.AluOpType.add,
                            )
                            r0 = blk * 256 + j * 128
                            nc.sync.dma_start(
                                out=out_d[r0:r0 + 128, h * w:(h + 1) * w],
                                in_=o_t[:, h * w:(h + 1) * w])
                            adds_done += 1
    nc.compile()  # bacc: splits multi-sem waits into EventSemaphore insts
    return nc


def _compose_r(thetas, rotation_pairs, theta_scale, rotation_matrix):
    """Replicates reference._compose_rotation."""
    idx = rotation_pairs.astype(np.int32)
    th = thetas.astype(np.float32) * np.float32(theta_scale[0])
    R = np.eye(D, dtype=np.float32)
    for k in range(th.shape[0]):
        i, j = int(idx[k, 0]), int(idx[k, 1])
        ck, sk = np.float32(np.cos(th[k])), np.float32(np.sin(th[k]))
        G = np.eye(D, dtype=np.float32)
        G[i, i] = ck
        G[i, j] = -sk
        G[j, i] = sk
        G[j, j] = ck
        R = (R @ G).astype(np.float32)
    return (R @ rotation_matrix.astype(np.float32)).astype(np.float32)


def _build_rr(R):
    """[128, 256] = [blockdiag(R2s,R2s) | blockdiag(R2sw,R2sw)] where
    R2s = [R_even|R_odd] (-> P = [u|v]) and R2sw = [-R_odd|R_even]
    (-> Q = [-v|u])."""
    r2s = np.concatenate([R[:, 0::2], R[:, 1::2]], axis=1)
    r2sw = np.concatenate([-R[:, 1::2], R[:, 0::2]], axis=1)

    def blkdiag(m):
        z = np.zeros((128, 128), dtype=np.float32)
        z[0:D, 0:D] = m
        z[D:128, D:128] = m
        return z

    return np.ascontiguousarray(
        np.concatenate([blkdiag(r2s), blkdiag(r2sw)], axis=1), dtype=np.float32)


def make_in_maps(x, thetas, rotation_pairs, theta_scale, rotation_matrix,
                 inv_freq):
    x = np.asarray(x, dtype=np.float32)
    R = _compose_r(
        np.asarray(thetas, np.float32),
        np.asarray(rotation_pairs, np.float32),
        np.asarray(theta_scale, np.float32),
        np.asarray(rotation_matrix, np.float32),
    )
    rr = _build_rr(R)
    invf = np.asarray(inv_freq, np.float32)
    pos = np.arange(S, dtype=np.float32)
    sinusoid = pos[:, None] * invf[None, :]               # [S, 32]
    cosf = np.cos(sinusoid).astype(np.float32)
    sinf = np.sin(sinusoid).astype(np.float32)

    in_maps = []
    for k in range(N_CORES):
        blk = slice(k * S_SH, (k + 1) * S_SH)
        # ccss[p, c, 0] = cos row, ccss[p, c, 1] = sin row (pos = c*128 + p)
        cc = cosf[blk].reshape(CBLK, 128, HALF)
        ss = sinf[blk].reshape(CBLK, 128, HALF)
        ccss = np.ascontiguousarray(
            np.stack([cc, ss], axis=2).transpose(1, 0, 2, 3), dtype=np.float32)
        xs = np.ascontiguousarray(x[:, blk, :]).reshape(ROWS, N_STATE)
        in_maps.append({"x": xs, "rr": rr, "ccss": ccss,
                        "ident": np.eye(128, dtype=np.float32)})
    return in_maps


def kernel(x, thetas, rotation_pairs, theta_scale, rotation_matrix, inv_freq):
    in_maps = make_in_maps(x, thetas, rotation_pairs, theta_scale,
                           rotation_matrix, inv_freq)
    if "nc" not in _compiled:
        _compiled["nc"] = _build_nc()
    res = run_bass_kernel_spmd(_compiled["nc"], in_maps,
                               list(range(N_CORES))).results

    out = np.empty((B, S, N_STATE), dtype=np.float32)
    for k in range(N_CORES):
        blk = slice(k * S_SH, (k + 1) * S_SH)
        out[:, blk, :] = res[k]["out"].reshape(B, S_SH, N_STATE)
    return out
